# revision 10
# baseline (speedup 1.0000x reference)
"""CenterLoss kernel for Trainium2 (8 NeuronCores, SPMD data-parallel).

Reference computes
    distmat[b,c] = ||x_b||^2 + ||c_c||^2 - 2<x_b, c_c>          [B, C]
    loss = sum(clip(distmat * onehot(labels), 1e-12, 1e12)) / B

Only distmat[b, labels[b]] survives the mask; each of the B*(C-1) masked
zeros becomes exactly 1e-12 under the clip. So instead of the [8192, 10000]
distmat (42 GFLOP), each core gathers its rows' centers with indirect DMA
and computes per-row squared distances; the host adds the closed-form
constant B*(C-1)*1e-12 and divides by B.

Sharding: batch split 8 ways (1024 rows/core), centers replicated.

Per-core kernel (all stock ISA):
  - one [128, 8] int32 idx tile (labels, row p*8+g at [p, g])
  - one contiguous 1MB x load as [128, 8, 256] (row p*8+g at [p, g, :])
  - 8 indirect row-gathers (Q7 SWDGE, 128 rows each) whose offset APs are
    column slices of the idx tile; compute pipelined behind them:
    DVE subtract, ACT Square with accum_out giving the row reduction.
  - [128, 8] partial distances DMA'd out; host clamps at 1e-12 and sums.

Hard-won HW constraints baked in here (this runtime rejects/crashes
otherwise):
  - Use Bacc, and finalize() before run: TRN2 codegen allows ONE sync-wait
    per instruction; Bacc's generate_event_semaphores splits multi-waits,
    and the bass2jax path serializes the module without finalizing.
  - Stock instructions only: custom "Ant" ISA ops (tensor_tensor_reduce,
    dma_gather, ...) kill the exec unit (NRT_EXEC_UNIT_UNRECOVERABLE).
  - No in-place DVE ops (out aliasing an input) — same crash.
  - indirect_dma_start: offset AP may be a [128, 1] column slice, but the
    dest must be a whole [128, D] tile; multi-column offsets or strided
    dest slices gather garbage.
"""

import ml_dtypes
import numpy as np

from concourse import bacc, bass, mybir
import concourse.tile as tile
from concourse.bass_utils import run_bass_kernel_spmd

B = 8192
C = 10000
D = 256
N_CORES = 8
BL = B // N_CORES  # rows per core
P = 128            # SBUF partitions
G = BL // P        # row groups per core

_CLIP_LO = 1e-12

_nc_cache = None


def _strip_dead_const_memsets(nc):
    """Bass.__init__ unconditionally memsets 4 const tiles (f32 0/1, bf16 1,
    u8 127). This kernel only uses immediate scalars, so they are dead code —
    and they sit before the first real instruction, so dropping them also
    drops them from the NEFF's measured span. Assert nothing references them
    before removing."""
    for func in nc.m.functions:
        for bb in func.blocks:
            for inst in bb.instructions:
                if type(inst).__name__ == "InstMemset":
                    continue
                for ap in list(inst.ins or []) + list(inst.outs or []):
                    memref = getattr(ap, "memref", "") or ""
                    assert not memref.startswith("const-"), (inst.name, memref)
    bb = nc.main_func.blocks[0]
    bb.instructions[:] = [
        inst
        for inst in bb.instructions
        if not (
            type(inst).__name__ == "InstMemset"
            and (inst.outs[0].memref or "").startswith("const-")
        )
    ]


def _build():
    global _nc_cache
    if _nc_cache is not None:
        return _nc_cache

    nc = bacc.Bacc()
    x_l = nc.dram_tensor("x_local", [BL, D], mybir.dt.bfloat16, kind="ExternalInput")
    lab_l = nc.dram_tensor("labels_local", [BL], mybir.dt.int32, kind="ExternalInput")
    cen = nc.dram_tensor("centers", [C, D], mybir.dt.bfloat16, kind="ExternalInput")
    out = nc.dram_tensor("partials", [P, G], mybir.dt.float32, kind="ExternalOutput")

    with tile.TileContext(nc) as tc:
        with (
            tc.tile_pool(name="big", bufs=1) as big,
            tc.tile_pool(name="work", bufs=4) as work,
            # gather dests get all 8 slots: late gathers then never carry a
            # slot-release wait, keeping the Q7 chain free of EVSEM stalls
            tc.tile_pool(name="ctp", bufs=G) as ctp,
        ):
            lt = big.tile([P, G], mybir.dt.int32)
            xt = big.tile([P, G, D], mybir.dt.bfloat16)
            acc = big.tile([P, G], mybir.dt.float32)

            # idx tile first: the whole gather chain hangs off it. Issue on
            # the Scalar engine's HWDGE queue — Sync spends ~700ns in its
            # entry drain while Scalar's sequencer is already free (compute
            # is all-DVE, so ACT has nothing else to do). The x loads go on
            # the SAME queue AFTER it so the 4KB labels transfer isn't stuck
            # behind 1MB of x on the shared DMA queues.
            nc.scalar.dma_start(out=lt[:], in_=lab_l[:].rearrange("(p g) -> p g", g=G))
            x_ap = x_l[:].rearrange("(p g) d -> p g d", g=G)
            nc.scalar.dma_start(out=xt[:, 0:G // 2, :], in_=x_ap[:, 0:G // 2, :])
            nc.scalar.dma_start(out=xt[:, G // 2:, :], in_=x_ap[:, G // 2:, :])

            # HW SWDGE generates ONE descriptor per offset-AP partition (128
            # max per instruction; extra offset columns are ignored), so the
            # gather is 8 serialized INDIRECT1Ds at ~1.4us each on the Pool
            # engine. Everything else overlaps behind that chain.
            for g in range(G):
                ct = ctp.tile([P, D], mybir.dt.bfloat16, tag="ct")
                nc.gpsimd.indirect_dma_start(
                    out=ct[:],
                    out_offset=None,
                    in_=cen[:],
                    in_offset=bass.IndirectOffsetOnAxis(ap=lt[:, g:g + 1], axis=0),
                )
                dt = work.tile([P, D], mybir.dt.bfloat16, tag="dt")
                nc.vector.tensor_sub(out=dt[:], in0=xt[:, g, :], in1=ct[:])
                sq = work.tile([P, D], mybir.dt.bfloat16, tag="sq")
                # (x-c)^2 + row-sum in one DVE op; ACT engine goes fully
                # unused (no act-table load, fewer const memsets)
                nc.vector.scalar_tensor_tensor(
                    out=sq[:],
                    in0=dt[:],
                    scalar=1.0,
                    in1=dt[:],
                    op0=mybir.AluOpType.mult,
                    op1=mybir.AluOpType.mult,
                    accum_out=acc[:, g:g + 1],
                )
                if g == G - 3:
                    # early out-DMA for finished groups: hides its ~1.3us
                    # completion latency behind the last groups' compute
                    nc.sync.dma_start(out=out[:, 0:G - 2], in_=acc[:, 0:G - 2])
            nc.sync.dma_start(out=out[:, G - 2:], in_=acc[:, G - 2:])

    _strip_dead_const_memsets(nc)

    nc.finalize()
    _nc_cache = nc
    return nc


def _run(x, labels, centers, **spmd_kwargs):
    nc = _build()
    # bf16 inputs halve DMA traffic; |rounding| ~0.4% per element averages
    # out over 8192 rows (net ~1e-5 on the loss, tolerance is 2e-2)
    x = np.ascontiguousarray(np.asarray(x), dtype=np.float32).astype(ml_dtypes.bfloat16)
    labels = np.ascontiguousarray(np.asarray(labels)).astype(np.int32)
    centers = np.ascontiguousarray(np.asarray(centers), dtype=np.float32).astype(
        ml_dtypes.bfloat16
    )

    in_maps = []
    for c in range(N_CORES):
        sl = slice(c * BL, (c + 1) * BL)
        in_maps.append(
            {
                "x_local": x[sl],
                "labels_local": labels[sl],
                "centers": centers,
            }
        )
    res = run_bass_kernel_spmd(nc, in_maps, list(range(N_CORES)), **spmd_kwargs)
    partials = np.stack([r["partials"] for r in res.results])  # [8, P, G]
    clamped = np.maximum(partials.astype(np.float64), _CLIP_LO)
    loss = (clamped.sum() + B * (C - 1) * _CLIP_LO) / B
    return np.asarray(loss, dtype=np.float32), res


def kernel(x, labels, centers):
    loss, _ = _run(x, labels, centers)
    return loss



# revision 11
# speedup vs baseline: 1.0069x; 1.0069x over previous
"""CenterLoss kernel for Trainium2 (8 NeuronCores, SPMD data-parallel).

Reference computes
    distmat[b,c] = ||x_b||^2 + ||c_c||^2 - 2<x_b, c_c>          [B, C]
    loss = sum(clip(distmat * onehot(labels), 1e-12, 1e12)) / B

Only distmat[b, labels[b]] survives the mask; each of the B*(C-1) masked
zeros becomes exactly 1e-12 under the clip. So instead of the [8192, 10000]
distmat (42 GFLOP), each core gathers its rows' centers with indirect DMA
and computes per-row squared distances; the host adds the closed-form
constant B*(C-1)*1e-12 and divides by B.

Sharding: batch split 8 ways (1024 rows/core), centers replicated.

Per-core kernel (all stock ISA):
  - one [128, 8] int32 idx tile (labels, row p*8+g at [p, g])
  - one contiguous 1MB x load as [128, 8, 256] (row p*8+g at [p, g, :])
  - 8 indirect row-gathers (Q7 SWDGE, 128 rows each) whose offset APs are
    column slices of the idx tile; compute pipelined behind them:
    DVE subtract, ACT Square with accum_out giving the row reduction.
  - [128, 8] partial distances DMA'd out; host clamps at 1e-12 and sums.

Hard-won HW constraints baked in here (this runtime rejects/crashes
otherwise):
  - Use Bacc, and finalize() before run: TRN2 codegen allows ONE sync-wait
    per instruction; Bacc's generate_event_semaphores splits multi-waits,
    and the bass2jax path serializes the module without finalizing.
  - Stock instructions only: custom "Ant" ISA ops (tensor_tensor_reduce,
    dma_gather, ...) kill the exec unit (NRT_EXEC_UNIT_UNRECOVERABLE).
  - No in-place DVE ops (out aliasing an input) — same crash.
  - indirect_dma_start: offset AP may be a [128, 1] column slice, but the
    dest must be a whole [128, D] tile; multi-column offsets or strided
    dest slices gather garbage.
"""

import ml_dtypes
import numpy as np

from concourse import bacc, bass, mybir
import concourse.tile as tile
from concourse.bass_utils import run_bass_kernel_spmd

B = 8192
C = 10000
D = 256
N_CORES = 8
BL = B // N_CORES  # rows per core
P = 128            # SBUF partitions
G = BL // P        # row groups per core

_CLIP_LO = 1e-12

_nc_cache = None


def _strip_dead_const_memsets(nc):
    """Bass.__init__ unconditionally memsets 4 const tiles (f32 0/1, bf16 1,
    u8 127). This kernel only uses immediate scalars, so they are dead code —
    and they sit before the first real instruction, so dropping them also
    drops them from the NEFF's measured span. Assert nothing references them
    before removing."""
    for func in nc.m.functions:
        for bb in func.blocks:
            for inst in bb.instructions:
                if type(inst).__name__ == "InstMemset":
                    continue
                for ap in list(inst.ins or []) + list(inst.outs or []):
                    memref = getattr(ap, "memref", "") or ""
                    assert not memref.startswith("const-"), (inst.name, memref)
    bb = nc.main_func.blocks[0]
    bb.instructions[:] = [
        inst
        for inst in bb.instructions
        if not (
            type(inst).__name__ == "InstMemset"
            and (inst.outs[0].memref or "").startswith("const-")
        )
    ]


def _build():
    global _nc_cache
    if _nc_cache is not None:
        return _nc_cache

    nc = bacc.Bacc()
    x_l = nc.dram_tensor("x_local", [BL, D], mybir.dt.bfloat16, kind="ExternalInput")
    lab_l = nc.dram_tensor("labels_local", [BL], mybir.dt.int32, kind="ExternalInput")
    cen = nc.dram_tensor("centers", [C, D], mybir.dt.bfloat16, kind="ExternalInput")
    out = nc.dram_tensor("partials", [P, G], mybir.dt.float32, kind="ExternalOutput")

    with tile.TileContext(nc) as tc:
        with (
            tc.tile_pool(name="big", bufs=1) as big,
            tc.tile_pool(name="work", bufs=4) as work,
            # gather dests get all 8 slots: late gathers then never carry a
            # slot-release wait, keeping the Q7 chain free of EVSEM stalls
            tc.tile_pool(name="ctp", bufs=G) as ctp,
        ):
            lt = big.tile([P, G], mybir.dt.int32)
            xt = big.tile([P, G, D], mybir.dt.bfloat16)
            acc = big.tile([P, G], mybir.dt.float32)

            # idx tile first: the whole gather chain hangs off it. Issue on
            # the Scalar engine's HWDGE queue — Sync spends ~700ns in its
            # entry drain while Scalar's sequencer is already free (compute
            # is all-DVE, so ACT has nothing else to do). The x loads go on
            # the SAME queue AFTER it so the 4KB labels transfer isn't stuck
            # behind 1MB of x on the shared DMA queues.
            nc.scalar.dma_start(out=lt[:], in_=lab_l[:].rearrange("(p g) -> p g", g=G))
            x_ap = x_l[:].rearrange("(p g) d -> p g d", g=G)
            nc.scalar.dma_start(out=xt[:, 0:G // 2, :], in_=x_ap[:, 0:G // 2, :])
            nc.scalar.dma_start(out=xt[:, G // 2:, :], in_=x_ap[:, G // 2:, :])

            # HW SWDGE generates ONE descriptor per offset-AP partition (128
            # max per instruction; extra offset columns are ignored), so the
            # gather is 8 serialized INDIRECT1Ds at ~1.4us each on the Pool
            # engine. Everything else overlaps behind that chain.
            for g in range(G):
                ct = ctp.tile([P, D], mybir.dt.bfloat16, tag="ct")
                nc.gpsimd.indirect_dma_start(
                    out=ct[:],
                    out_offset=None,
                    in_=cen[:],
                    in_offset=bass.IndirectOffsetOnAxis(ap=lt[:, g:g + 1], axis=0),
                )
                dt = work.tile([P, D], mybir.dt.bfloat16, tag="dt")
                nc.vector.tensor_sub(out=dt[:], in0=xt[:, g, :], in1=ct[:])
                sq = work.tile([P, D], mybir.dt.bfloat16, tag="sq")
                # (x-c)^2 + row-sum in one DVE op; ACT engine goes fully
                # unused (no act-table load, fewer const memsets)
                nc.vector.scalar_tensor_tensor(
                    out=sq[:],
                    in0=dt[:],
                    scalar=1.0,
                    in1=dt[:],
                    op0=mybir.AluOpType.mult,
                    op1=mybir.AluOpType.mult,
                    accum_out=acc[:, g:g + 1],
                )
                if g == G - 3:
                    # early out-DMA for finished groups: hides its ~1.3us
                    # completion latency behind the last groups' compute
                    nc.scalar.dma_start(out=out[:, 0:G - 2], in_=acc[:, 0:G - 2])
            nc.scalar.dma_start(out=out[:, G - 2:], in_=acc[:, G - 2:])

    _strip_dead_const_memsets(nc)

    nc.finalize()
    _nc_cache = nc
    return nc


def _run(x, labels, centers, **spmd_kwargs):
    nc = _build()
    # bf16 inputs halve DMA traffic; |rounding| ~0.4% per element averages
    # out over 8192 rows (net ~1e-5 on the loss, tolerance is 2e-2)
    x = np.ascontiguousarray(np.asarray(x), dtype=np.float32).astype(ml_dtypes.bfloat16)
    labels = np.ascontiguousarray(np.asarray(labels)).astype(np.int32)
    centers = np.ascontiguousarray(np.asarray(centers), dtype=np.float32).astype(
        ml_dtypes.bfloat16
    )

    in_maps = []
    for c in range(N_CORES):
        sl = slice(c * BL, (c + 1) * BL)
        in_maps.append(
            {
                "x_local": x[sl],
                "labels_local": labels[sl],
                "centers": centers,
            }
        )
    res = run_bass_kernel_spmd(nc, in_maps, list(range(N_CORES)), **spmd_kwargs)
    partials = np.stack([r["partials"] for r in res.results])  # [8, P, G]
    clamped = np.maximum(partials.astype(np.float64), _CLIP_LO)
    loss = (clamped.sum() + B * (C - 1) * _CLIP_LO) / B
    return np.asarray(loss, dtype=np.float32), res


def kernel(x, labels, centers):
    loss, _ = _run(x, labels, centers)
    return loss

